# revision 2
# baseline (speedup 1.0000x reference)
"""GCN message passing (SpMM) on 8 Trainium2 NeuronCores — v4.

out[r, :] = sum_{e: rows[e]==r} vals[e] * x[cols[e], :]

v4 changes vs v3 (all driven by HW microbenchmarks; the CoreSim cost model
underestimates HW by 4-10x on every component):
  - Gathers spread across 4 SWDGE queues (per-queue ring drain is the HW
    bottleneck: 1 queue ~6ns/idx, 4 queues ~2ns/idx).
  - x stored as [N, 128] bf16 (256B tokens, the gather minimum); matmuls
    run bf16 x bf16 -> f32 PSUM (fp32 matmul is PE-rate-limited ~2.4x
    slower on HW).
  - S matrices (val-weighted one-hot segment selectors) are precomputed on
    the host and streamed per chunk via HWDGE — eliminates the DVE
    is_equal/mult build entirely (530us on HW).
  - scatter_add pinned to queue 3 (sharing rotating-queue rings with the
    gathers convoys them: RMW descriptors block the ring), bf16 tokens
    (128B payload, 256B stride), dump rows spread over 64 slots to avoid
    RMW hot-spotting one address.
Structure kept from v3: sorted rows => core k owns rows [k*12500,(k+1)*12500);
greedy windows (<=32 consecutive rows, <=128 edges per node-bucket), chunks
of CW=30 windows = one PSUM pass, 4 dma_gathers + 120 matmuls + 1 ACT copy
+ 1 dma_scatter_add per chunk, 4 rotating output slabs summed on the host.
"""

import numpy as np

import concourse.bass as bass
import concourse.bacc as bacc
import concourse.mybir as mybir
import concourse.tile as tile
from concourse.bass_utils import run_bass_kernel_spmd

# ---------------- problem constants (hardcoded per the task contract) -------
N_NODES = 100000
D = 48
N_CORES = 8
R_PER_CORE = N_NODES // N_CORES  # 12500

# ---------------- kernel hyperparameters -----------------------------------
NB = 4               # node-range buckets (int16 gather indices: 25000 < 32768)
B_NODES = N_NODES // NB
EDGE_CAP = 128       # edges per (window, bucket) tile = PE contraction dim
SEG_CAP = 32         # max rows per window (= matmul M, psum partition group)
GP = 3               # usable 32-partition psum groups
CW = 30              # windows per chunk (= one PSUM pass: 3 groups x 10)
SC_H = CW // GP      # free blocks per pass (10)
EL = 64              # out-slab row payload, bf16 elements (128B)
EST = 128            # out-slab row stride, bf16 elements (256B)
NDUMP = 64           # dump rows spread to avoid RMW hot-spotting
XT = 128             # x token width, bf16 elements (256B)
DUMP = R_PER_CORE    # dump row index in the out slabs
NQ = 4               # SWDGE queues

_F32 = mybir.dt.float32
_BF16 = mybir.dt.bfloat16
_I16 = mybir.dt.int16

_NIG = CW * EDGE_CAP          # gather indices per (chunk, bucket) = 3840
_NIS = 128 * SC_H             # scatter indices per chunk = 1280
_GI_W = _NIG // 16            # 240 int16 per partition per chunk
_SI_W = _NIS // 16            # 80
_SW = CW * NB * SEG_CAP       # S elems per chunk per partition = 3840


def _wrap16(flat, reps=8):
    """[(n)] int16 -> [16*reps, n/16] in the 16-partition wrap, replicated."""
    n = flat.shape[0]
    w = flat.reshape(n // 16, 16).T  # [16, n/16]
    return np.tile(w, (reps, 1))


def _to_bf16(a):
    """f32 ndarray -> bf16 (ml_dtypes) by round-to-nearest-even."""
    import ml_dtypes
    return a.astype(ml_dtypes.bfloat16)


# ===========================================================================
# Host-side prep: pure index/layout transformation.
# ===========================================================================
def _pack_core(rows_l, cols, vals, r_per_core):
    bucket = (cols // B_NODES).astype(np.int64)
    col_loc = (cols - bucket * B_NODES).astype(np.int16)

    cnt = np.zeros((r_per_core, NB), np.int64)
    np.add.at(cnt, (rows_l, bucket), 1)
    assert cnt.max() <= EDGE_CAP, "row degree exceeds tile capacity"

    # greedy windows over consecutive rows: <=SEG_CAP rows, <=EDGE_CAP
    # edges per bucket per window
    window_of_row = np.empty(r_per_core, np.int64)
    slot_of_row = np.empty(r_per_core, np.int64)
    w = 0
    acc = np.zeros(NB, np.int64)
    nrows = 0
    for r in range(r_per_core):
        c = cnt[r]
        if nrows == SEG_CAP or (acc + c > EDGE_CAP).any():
            w += 1
            acc[:] = 0
            nrows = 0
        window_of_row[r] = w
        slot_of_row[r] = nrows
        acc += c
        nrows += 1
    n_win = w + 1

    w_e = window_of_row[rows_l]
    slot_e = slot_of_row[rows_l]

    per_bucket = []
    for b in range(NB):
        sel = np.flatnonzero(bucket == b)
        o = np.argsort(w_e[sel], kind="stable")
        sel = sel[o]
        wb = w_e[sel]                       # non-decreasing after sort
        first = np.searchsorted(wb, np.arange(n_win))
        pos = np.arange(sel.shape[0]) - first[wb]
        assert pos.max(initial=0) < EDGE_CAP
        colb = np.zeros((n_win, EDGE_CAP), np.int16)
        valb = np.zeros((n_win, EDGE_CAP), np.float32)
        slotb = np.zeros((n_win, EDGE_CAP), np.int64)
        colb[wb, pos] = col_loc[sel]
        valb[wb, pos] = vals[sel]
        slotb[wb, pos] = slot_e[sel]
        per_bucket.append((colb, valb, slotb))

    sidx = np.full((n_win, SEG_CAP), DUMP, np.int16)
    sidx[window_of_row, slot_of_row] = np.arange(r_per_core, dtype=np.int16)
    return per_bucket, sidx, n_win


def prep_inputs(adj_rows, adj_cols, adj_vals):
    """Shard + pack. Returns (per-core in_map list, n_chunks)."""
    adj_rows = np.asarray(adj_rows).astype(np.int64)
    adj_cols = np.asarray(adj_cols).astype(np.int64)
    adj_vals = np.asarray(adj_vals).astype(np.float32)

    bounds = np.searchsorted(adj_rows, np.arange(N_CORES + 1) * R_PER_CORE)
    packed = []
    for k in range(N_CORES):
        e0, e1 = bounds[k], bounds[k + 1]
        rows_l = adj_rows[e0:e1] - k * R_PER_CORE
        packed.append(_pack_core(rows_l, adj_cols[e0:e1],
                                 adj_vals[e0:e1], R_PER_CORE))

    nw_max = max(p[2] for p in packed)
    nw_pad = -(-nw_max // CW) * CW
    n_chunks = nw_pad // CW

    s_iota = np.arange(SEG_CAP, dtype=np.int64)
    in_maps = []
    for k in range(N_CORES):
        per_bucket, sidx, n_win = packed[k]
        m = {"zeros": _to_bf16(np.zeros((128, SC_H * EL), np.float32))}
        # S tiles: [nw_pad, NB, 128 pos, 32 slot] bf16, val-weighted one-hot.
        s_all = np.zeros((nw_pad, NB, EDGE_CAP, SEG_CAP), np.float32)
        for b in range(NB):
            colb, valb, slotb = per_bucket[b]
            cb = np.zeros((nw_pad, EDGE_CAP), np.int16)
            cb[:n_win] = colb
            # gidx: [128, n_chunks*_GI_W] int16 (16-wrap per chunk, x8)
            m[f"gidx{b}"] = np.concatenate([
                _wrap16(cb[c * CW:(c + 1) * CW].reshape(-1))
                for c in range(n_chunks)], axis=1)
            s_all[:n_win, b] = (valb[:, :, None]
                                * (slotb[:, :, None] == s_iota[None, None, :]))
        # s: [128 pos, n_chunks*_SW] bf16; tile (c, wl, b) at free offset
        # ((c*CW + wl)*NB + b)*SEG_CAP
        m["sS"] = np.ascontiguousarray(
            _to_bf16(s_all.reshape(n_chunks, CW * NB, EDGE_CAP, SEG_CAP)
                     .transpose(2, 0, 1, 3).reshape(128, n_chunks * _SW)))
        st = np.full((nw_pad, SEG_CAP), DUMP, np.int16)
        st[:n_win] = sidx
        # scatter idx: slot i -> (p=i%128, j=i//128); p=32a+s (a<3), w=c*CW+3j+a
        sflat = (DUMP + (np.arange(128, dtype=np.int16) % NDUMP)
                 * np.ones((n_chunks, SC_H, 1), np.int16))  # [c, j, p]
        w_idx = (np.arange(n_chunks * CW).reshape(n_chunks, CW)
                 .reshape(n_chunks, SC_H, GP))  # [c, j, a] -> w = c*CW+3j+a
        for a in range(GP):
            sflat[:, :, 32 * a:32 * (a + 1)] = st[w_idx[:, :, a]]
        m["sidx"] = np.concatenate([
            _wrap16(sflat[c].reshape(-1)) for c in range(n_chunks)], axis=1)
        in_maps.append(m)
    return in_maps, n_chunks


def pad_x_bf16(x):
    xb = np.zeros((N_NODES, XT), np.float32)
    xb[:, :D] = x
    return _to_bf16(xb)


# ===========================================================================
# Device program (shared across all 8 cores)
# ===========================================================================
def build_program(n_chunks, repeat=1, scatter_rotate=False):
    nc = bacc.Bacc("TRN2", target_bir_lowering=False, debug=False,
                   num_devices=N_CORES, num_swdge_queues=NQ)
    x_d = nc.dram_tensor("xbf", [N_NODES, XT], _BF16, kind="ExternalInput")
    gidx_d = [nc.dram_tensor(f"gidx{b}", [128, n_chunks * _GI_W], _I16,
                             kind="ExternalInput") for b in range(NB)]
    s_d = nc.dram_tensor("sS", [128, n_chunks * _SW], _BF16,
                         kind="ExternalInput")
    sidx_d = nc.dram_tensor("sidx", [128, n_chunks * _SI_W], _I16,
                            kind="ExternalInput")
    zeros_d = nc.dram_tensor("zeros", [128, SC_H * EL], _BF16,
                             kind="ExternalInput")
    out_d = [nc.dram_tensor(f"out{h}", [R_PER_CORE + NDUMP, EST], _BF16,
                            kind="ExternalOutput") for h in range(4)]

    with tile.TileContext(nc) as tc:
        with (
            tc.tile_pool(name="meta", bufs=1) as meta,
            tc.tile_pool(name="gbuf", bufs=2) as gbuf,
            tc.tile_pool(name="sbuf_s", bufs=3) as sbuf_s,
            tc.tile_pool(name="psum", bufs=3, space="PSUM") as psum,
        ):
            gi_all = []
            for b in range(NB):
                gi = meta.tile([128, n_chunks * _GI_W], _I16, tag=f"giA{b}")
                nc.sync.dma_start(out=gi[:], in_=gidx_d[b][:])
                gi_all.append(gi)
            si_all = meta.tile([128, n_chunks * _SI_W], _I16, tag="siA")
            nc.sync.dma_start(out=si_all[:], in_=sidx_d[:])
            sc_ts = []
            for h in range(4):
                sc = meta.tile([128, SC_H * EL], _BF16, tag=f"scA{h}")
                nc.sync.dma_start(out=sc[:], in_=zeros_d[:])
                sc_ts.append(sc)

            for c in [c for _ in range(repeat) for c in range(n_chunks)]:
                s_t = sbuf_s.tile([128, _SW], _BF16, tag="s")
                nc.sync.dma_start(out=s_t[:],
                                  in_=s_d[:, c * _SW:(c + 1) * _SW])
                g_ts = []
                for b in range(NB):
                    g_t = gbuf.tile([128, CW * XT], _BF16, tag=f"g{b}")
                    nc.gpsimd.dma_gather(
                        out_ap=g_t[:].rearrange("p (t f) -> p t f", f=XT),
                        in_ap=x_d[B_NODES * b:B_NODES * (b + 1)],
                        idxs_ap=gi_all[b][:, c * _GI_W:(c + 1) * _GI_W],
                        num_idxs=_NIG, num_idxs_reg=_NIG, elem_size=XT,
                        single_packet=False, queue_num=b % NQ,
                    )
                    g_ts.append(g_t)

                ps = psum.tile([128, SC_H * D], _F32, space="PSUM", tag="ps")
                for wl in range(CW):
                    a, j = wl % GP, wl // GP
                    for b in range(NB):
                        nc.tensor.matmul(
                            out=ps[32 * a:32 * a + SEG_CAP, D * j:D * j + D],
                            lhsT=s_t[:, (wl * NB + b) * SEG_CAP:
                                     (wl * NB + b + 1) * SEG_CAP],
                            rhs=g_ts[b][:, XT * wl:XT * wl + D],
                            start=(b == 0), stop=(b == NB - 1),
                            skip_group_check=True,
                        )

                sc_t = sc_ts[c % 4]
                sc3 = sc_t[:].rearrange("p (j f) -> p j f", f=EL)
                ps3 = ps[:].rearrange("p (j f) -> p j f", f=D)
                nc.scalar.copy(out=sc3[:96, :, :D], in_=ps3[:96])
                nc.gpsimd.dma_scatter_add(
                    out_d[c % 4][:, :EL],
                    sc3[:],
                    si_all[:, c * _SI_W:(c + 1) * _SI_W],
                    num_idxs=_NIS, num_idxs_reg=_NIS, elem_size=EL,
                    elem_step=EST,
                    single_packet=False,
                    queue_num=(c % NQ) if scatter_rotate else NQ - 1,
                )
    nc.compile()
    return nc


# ===========================================================================
# Entry point
# ===========================================================================
_CACHE = {}


def _get_program(n_chunks):
    if n_chunks not in _CACHE:
        _CACHE[n_chunks] = build_program(n_chunks)
    return _CACHE[n_chunks]


def _run(adj_rows, adj_cols, adj_vals, x):
    xbf = pad_x_bf16(np.ascontiguousarray(np.asarray(x), dtype=np.float32))
    in_maps, n_chunks = prep_inputs(adj_rows, adj_cols, adj_vals)
    for m in in_maps:
        m["xbf"] = xbf
    nc = _get_program(n_chunks)
    res = run_bass_kernel_spmd(nc, in_maps, core_ids=list(range(N_CORES)))
    out = np.empty((N_NODES, D), np.float32)
    for k in range(N_CORES):
        slab = sum(res.results[k][f"out{h}"][:R_PER_CORE, :D]
                   .astype(np.float32) for h in range(4))
        out[k * R_PER_CORE:(k + 1) * R_PER_CORE] = slab
    return out, res, (in_maps, n_chunks)


def kernel(adj_rows, adj_cols, adj_vals, x):
    out, _, _ = _run(adj_rows, adj_cols, adj_vals, x)
    return out


# revision 3
# speedup vs baseline: 1.1417x; 1.1417x over previous
"""GCN message passing (SpMM) on 8 Trainium2 NeuronCores — v4.

out[r, :] = sum_{e: rows[e]==r} vals[e] * x[cols[e], :]

v4 changes vs v3 (all driven by HW microbenchmarks; the CoreSim cost model
underestimates HW by 4-10x on every component):
  - Gathers spread across 4 SWDGE queues (per-queue ring drain is the HW
    bottleneck: 1 queue ~6ns/idx, 4 queues ~2ns/idx).
  - x stored as [N, 128] bf16 (256B tokens, the gather minimum); matmuls
    run bf16 x bf16 -> f32 PSUM (fp32 matmul is PE-rate-limited ~2.4x
    slower on HW).
  - S matrices (val-weighted one-hot segment selectors) are precomputed on
    the host and streamed per chunk via HWDGE — eliminates the DVE
    is_equal/mult build entirely (530us on HW).
  - scatter_add pinned to queue 3 (sharing rotating-queue rings with the
    gathers convoys them: RMW descriptors block the ring), bf16 tokens
    (128B payload, 256B stride), dump rows spread over 64 slots to avoid
    RMW hot-spotting one address.
Structure kept from v3: sorted rows => core k owns rows [k*12500,(k+1)*12500);
greedy windows (<=32 consecutive rows, <=128 edges per node-bucket), chunks
of CW=30 windows = one PSUM pass, 4 dma_gathers + 120 matmuls + 1 ACT copy
+ 1 dma_scatter_add per chunk, 4 rotating output slabs summed on the host.
"""

import numpy as np

import concourse.bass as bass
import concourse.bacc as bacc
import concourse.mybir as mybir
import concourse.tile as tile
from concourse.bass_utils import run_bass_kernel_spmd

# ---------------- problem constants (hardcoded per the task contract) -------
N_NODES = 100000
D = 48
N_CORES = 8
R_PER_CORE = N_NODES // N_CORES  # 12500

# ---------------- kernel hyperparameters -----------------------------------
NB = 4               # node-range buckets (int16 gather indices: 25000 < 32768)
B_NODES = N_NODES // NB
EDGE_CAP = 128       # edges per (window, bucket) tile = PE contraction dim
SEG_CAP = 32         # max rows per window (= matmul M, psum partition group)
GP = 4               # 32-partition psum groups per pass (all 128 partitions)
CW = 40              # windows per chunk (= one PSUM pass: 4 groups x 10)
SC_H = CW // GP      # free blocks per pass (10)
EL = 64              # out-slab row payload, bf16 elements (128B)
EST = 128            # out-slab row stride, bf16 elements (256B)
NDUMP = 64           # dump rows spread to avoid RMW hot-spotting
XT = 128             # x token width, bf16 elements (256B)
DUMP = R_PER_CORE    # dump row index in the out slabs
NQ = 4               # SWDGE queues

_F32 = mybir.dt.float32
_BF16 = mybir.dt.bfloat16
_I16 = mybir.dt.int16

_NIG = CW * EDGE_CAP          # gather indices per (chunk, bucket) = 3840
_NIS = 128 * SC_H             # scatter indices per chunk = 1280
_GI_W = _NIG // 16            # 240 int16 per partition per chunk
_SI_W = _NIS // 16            # 80
_SW = CW * NB * SEG_CAP       # S elems per chunk per partition = 3840


def _wrap16(flat, reps=8):
    """[(n)] int16 -> [16*reps, n/16] in the 16-partition wrap, replicated."""
    n = flat.shape[0]
    w = flat.reshape(n // 16, 16).T  # [16, n/16]
    return np.tile(w, (reps, 1))


def _to_bf16(a):
    """f32 ndarray -> bf16 (ml_dtypes) by round-to-nearest-even."""
    import ml_dtypes
    return a.astype(ml_dtypes.bfloat16)


# ===========================================================================
# Host-side prep: pure index/layout transformation.
# ===========================================================================
def _pack_core(rows_l, cols, vals, r_per_core):
    bucket = (cols // B_NODES).astype(np.int64)
    col_loc = (cols - bucket * B_NODES).astype(np.int16)

    cnt = np.zeros((r_per_core, NB), np.int64)
    np.add.at(cnt, (rows_l, bucket), 1)
    assert cnt.max() <= EDGE_CAP, "row degree exceeds tile capacity"

    # greedy windows over consecutive rows: <=SEG_CAP rows, <=EDGE_CAP
    # edges per bucket per window
    window_of_row = np.empty(r_per_core, np.int64)
    slot_of_row = np.empty(r_per_core, np.int64)
    w = 0
    acc = np.zeros(NB, np.int64)
    nrows = 0
    for r in range(r_per_core):
        c = cnt[r]
        if nrows == SEG_CAP or (acc + c > EDGE_CAP).any():
            w += 1
            acc[:] = 0
            nrows = 0
        window_of_row[r] = w
        slot_of_row[r] = nrows
        acc += c
        nrows += 1
    n_win = w + 1

    w_e = window_of_row[rows_l]
    slot_e = slot_of_row[rows_l]

    per_bucket = []
    for b in range(NB):
        sel = np.flatnonzero(bucket == b)
        o = np.argsort(w_e[sel], kind="stable")
        sel = sel[o]
        wb = w_e[sel]                       # non-decreasing after sort
        first = np.searchsorted(wb, np.arange(n_win))
        pos = np.arange(sel.shape[0]) - first[wb]
        assert pos.max(initial=0) < EDGE_CAP
        colb = np.zeros((n_win, EDGE_CAP), np.int16)
        valb = np.zeros((n_win, EDGE_CAP), np.float32)
        slotb = np.zeros((n_win, EDGE_CAP), np.int64)
        colb[wb, pos] = col_loc[sel]
        valb[wb, pos] = vals[sel]
        slotb[wb, pos] = slot_e[sel]
        per_bucket.append((colb, valb, slotb))

    sidx = np.full((n_win, SEG_CAP), DUMP, np.int16)
    sidx[window_of_row, slot_of_row] = np.arange(r_per_core, dtype=np.int16)
    return per_bucket, sidx, n_win


def prep_inputs(adj_rows, adj_cols, adj_vals):
    """Shard + pack. Returns (per-core in_map list, n_chunks)."""
    adj_rows = np.asarray(adj_rows).astype(np.int64)
    adj_cols = np.asarray(adj_cols).astype(np.int64)
    adj_vals = np.asarray(adj_vals).astype(np.float32)

    bounds = np.searchsorted(adj_rows, np.arange(N_CORES + 1) * R_PER_CORE)
    packed = []
    for k in range(N_CORES):
        e0, e1 = bounds[k], bounds[k + 1]
        rows_l = adj_rows[e0:e1] - k * R_PER_CORE
        packed.append(_pack_core(rows_l, adj_cols[e0:e1],
                                 adj_vals[e0:e1], R_PER_CORE))

    nw_max = max(p[2] for p in packed)
    nw_pad = -(-nw_max // CW) * CW
    n_chunks = nw_pad // CW

    s_iota = np.arange(SEG_CAP, dtype=np.int64)
    in_maps = []
    for k in range(N_CORES):
        per_bucket, sidx, n_win = packed[k]
        m = {"zeros": _to_bf16(np.zeros((128, SC_H * EL), np.float32))}
        # S tiles: [nw_pad, NB, 128 pos, 32 slot] bf16, val-weighted one-hot.
        s_all = np.zeros((nw_pad, NB, EDGE_CAP, SEG_CAP), np.float32)
        for b in range(NB):
            colb, valb, slotb = per_bucket[b]
            cb = np.zeros((nw_pad, EDGE_CAP), np.int16)
            cb[:n_win] = colb
            # gidx: [128, n_chunks*_GI_W] int16 (16-wrap per chunk, x8)
            m[f"gidx{b}"] = np.concatenate([
                _wrap16(cb[c * CW:(c + 1) * CW].reshape(-1))
                for c in range(n_chunks)], axis=1)
            s_all[:n_win, b] = (valb[:, :, None]
                                * (slotb[:, :, None] == s_iota[None, None, :]))
        # s: [128 pos, n_chunks*_SW] bf16; tile (c, wl, b) at free offset
        # ((c*CW + wl)*NB + b)*SEG_CAP
        m["sS"] = np.ascontiguousarray(
            _to_bf16(s_all.reshape(n_chunks, CW * NB, EDGE_CAP, SEG_CAP)
                     .transpose(2, 0, 1, 3).reshape(128, n_chunks * _SW)))
        st = np.full((nw_pad, SEG_CAP), DUMP, np.int16)
        st[:n_win] = sidx
        # scatter idx: slot i -> (p=i%128, j=i//128); p=32a+s (a<3), w=c*CW+3j+a
        sflat = (DUMP + (np.arange(128, dtype=np.int16) % NDUMP)
                 * np.ones((n_chunks, SC_H, 1), np.int16))  # [c, j, p]
        w_idx = (np.arange(n_chunks * CW).reshape(n_chunks, CW)
                 .reshape(n_chunks, SC_H, GP))  # [c, j, a] -> w = c*CW+3j+a
        for a in range(GP):
            sflat[:, :, 32 * a:32 * (a + 1)] = st[w_idx[:, :, a]]
        m["sidx"] = np.concatenate([
            _wrap16(sflat[c].reshape(-1)) for c in range(n_chunks)], axis=1)
        in_maps.append(m)
    return in_maps, n_chunks


def pad_x_bf16(x):
    xb = np.zeros((N_NODES, XT), np.float32)
    xb[:, :D] = x
    return _to_bf16(xb)


# ===========================================================================
# Device program (shared across all 8 cores)
# ===========================================================================
def build_program(n_chunks, repeat=1, scatter_rotate=False):
    nc = bacc.Bacc("TRN2", target_bir_lowering=False, debug=False,
                   num_devices=N_CORES, num_swdge_queues=NQ)
    x_d = nc.dram_tensor("xbf", [N_NODES, XT], _BF16, kind="ExternalInput")
    gidx_d = [nc.dram_tensor(f"gidx{b}", [128, n_chunks * _GI_W], _I16,
                             kind="ExternalInput") for b in range(NB)]
    s_d = nc.dram_tensor("sS", [128, n_chunks * _SW], _BF16,
                         kind="ExternalInput")
    sidx_d = nc.dram_tensor("sidx", [128, n_chunks * _SI_W], _I16,
                            kind="ExternalInput")
    zeros_d = nc.dram_tensor("zeros", [128, SC_H * EL], _BF16,
                             kind="ExternalInput")
    out_d = [nc.dram_tensor(f"out{h}", [R_PER_CORE + NDUMP, EST], _BF16,
                            kind="ExternalOutput") for h in range(4)]

    with tile.TileContext(nc) as tc:
        with (
            tc.tile_pool(name="meta", bufs=1) as meta,
            tc.tile_pool(name="gbuf", bufs=3) as gbuf,
            tc.tile_pool(name="sbuf_s", bufs=3) as sbuf_s,
            tc.tile_pool(name="psum", bufs=3, space="PSUM") as psum,
        ):
            gi_all = []
            for b in range(NB):
                gi = meta.tile([128, n_chunks * _GI_W], _I16, tag=f"giA{b}")
                nc.sync.dma_start(out=gi[:], in_=gidx_d[b][:])
                gi_all.append(gi)
            si_all = meta.tile([128, n_chunks * _SI_W], _I16, tag="siA")
            nc.sync.dma_start(out=si_all[:], in_=sidx_d[:])
            sc_ts = []
            for h in range(4):
                sc = meta.tile([128, SC_H * EL], _BF16, tag=f"scA{h}")
                nc.sync.dma_start(out=sc[:], in_=zeros_d[:])
                sc_ts.append(sc)

            for c in [c for _ in range(repeat) for c in range(n_chunks)]:
                s_t = sbuf_s.tile([128, _SW], _BF16, tag="s")
                nc.sync.dma_start(out=s_t[:],
                                  in_=s_d[:, c * _SW:(c + 1) * _SW])
                g_ts = []
                for b in range(NB):
                    g_t = gbuf.tile([128, CW * XT], _BF16, tag=f"g{b}")
                    nc.gpsimd.dma_gather(
                        out_ap=g_t[:].rearrange("p (t f) -> p t f", f=XT),
                        in_ap=x_d[B_NODES * b:B_NODES * (b + 1)],
                        idxs_ap=gi_all[b][:, c * _GI_W:(c + 1) * _GI_W],
                        num_idxs=_NIG, num_idxs_reg=_NIG, elem_size=XT,
                        single_packet=False, queue_num=b % NQ,
                    )
                    g_ts.append(g_t)

                ps = psum.tile([128, SC_H * D], _F32, space="PSUM", tag="ps")
                for wl in range(CW):
                    a, j = wl % GP, wl // GP
                    for b in range(NB):
                        nc.tensor.matmul(
                            out=ps[32 * a:32 * a + SEG_CAP, D * j:D * j + D],
                            lhsT=s_t[:, (wl * NB + b) * SEG_CAP:
                                     (wl * NB + b + 1) * SEG_CAP],
                            rhs=g_ts[b][:, XT * wl:XT * wl + D],
                            start=(b == 0), stop=(b == NB - 1),
                            skip_group_check=True,
                            tile_position=(0, 32 * a),
                        )

                sc_t = sc_ts[c % 4]
                sc3 = sc_t[:].rearrange("p (j f) -> p j f", f=EL)
                ps3 = ps[:].rearrange("p (j f) -> p j f", f=D)
                nc.scalar.copy(out=sc3[:, :, :D], in_=ps3[:])
                nc.gpsimd.dma_scatter_add(
                    out_d[c % 4][:, :EL],
                    sc3[:],
                    si_all[:, c * _SI_W:(c + 1) * _SI_W],
                    num_idxs=_NIS, num_idxs_reg=_NIS, elem_size=EL,
                    elem_step=EST,
                    single_packet=False,
                    queue_num=(c % NQ) if scatter_rotate else NQ - 1,
                )
    nc.compile()
    return nc


# ===========================================================================
# Entry point
# ===========================================================================
_CACHE = {}


def _get_program(n_chunks):
    if n_chunks not in _CACHE:
        _CACHE[n_chunks] = build_program(n_chunks)
    return _CACHE[n_chunks]


def _run(adj_rows, adj_cols, adj_vals, x):
    xbf = pad_x_bf16(np.ascontiguousarray(np.asarray(x), dtype=np.float32))
    in_maps, n_chunks = prep_inputs(adj_rows, adj_cols, adj_vals)
    for m in in_maps:
        m["xbf"] = xbf
    nc = _get_program(n_chunks)
    res = run_bass_kernel_spmd(nc, in_maps, core_ids=list(range(N_CORES)))
    out = np.empty((N_NODES, D), np.float32)
    for k in range(N_CORES):
        slab = sum(res.results[k][f"out{h}"][:R_PER_CORE, :D]
                   .astype(np.float32) for h in range(4))
        out[k * R_PER_CORE:(k + 1) * R_PER_CORE] = slab
    return out, res, (in_maps, n_chunks)


def kernel(adj_rows, adj_cols, adj_vals, x):
    out, _, _ = _run(adj_rows, adj_cols, adj_vals, x)
    return out
